# revision 35
# baseline (speedup 1.0000x reference)
"""Chamfer distance loss kernel for Trainium2 (8 NeuronCores).

Problem: B=4, N=8192, C=3. loss = mean_i min_j d[i,j] + mean_j min_i d[i,j]
over per-batch 8192x8192 squared-distance matrices.

Strategy:
  - 8 cores: core c handles batch c//2, target-row half c%2 (4096 rows x 8192 cols).
  - Host pre-augments inputs so the PE produces the distance matrix directly:
      d = lhsT.T @ rhs,  d[i,j] = |t_i|^2 + |x_j|^2 - 2 t_i.x_j
    Values are hi/lo bf16-split (v = hi + lo) and the product expanded to
    hi*hi + lo*hi + hi*lo (lo*lo dropped, ~2^-18 relative) -> K=13 bf16
    matmul with near-fp32 accuracy at 1 cycle/row.
  - Per row-block b (128 rows): 4 PSUM tiles [128,2048] are drained into one
    contiguous dtile [128,8192] bf16. Most quarters drain via ScalarE
    activation-copy; a few via VectorE tensor_copy to balance engine load.
  - VectorE per block: one tensor_scalar (4x mode, FD 8192) computes the
    row-min via accum_out; one tensor_tensor (2x mode, FD 8192) min-merges
    the dtile into colacc.
  - Last block's TT + colacc DMA are chunked to hide the output-DMA tail.
  - dist1 [128, nb] row-mins exact to bf16(d); colacc [128, 8192] partial
    col-mins folded on host (min over partitions + halves).
"""

import numpy as np

N_CORES = 8
P = 128
MM_N = 512  # matmul moving free width (one PSUM bank of f32 output)
QW = 2048  # PSUM tile width (4 banks of f32); 2 tiles ping-pong
FLT_BIG = 3.0e38
K_AUG = 13  # hi/lo-split augmented contraction depth

# Tail columns of each block's last quarter drained by VectorE (fused with a
# row-min accumulator) instead of ScalarE, balancing per-block engine load:
# ACT 3*2048+(2048-VW) cols vs DVE drain(VW) + rowmin TS + colmin TT.
VW = 576
# The last E_RAW blocks skip the on-device col-min merge; their drained
# tiles are DMAed out raw and folded into the col-min on the host. This
# removes VectorE work from the pipeline tail (nothing can overlap it).
E_RAW = 2

_NC_CACHE = {}


def _build(rows, ncols):
    import concourse.bacc as bacc
    import concourse.mybir as mybir
    from concourse.tile import TileContext
    from contextlib import ExitStack

    f32 = mybir.dt.float32
    bf16 = mybir.dt.bfloat16
    MIN = mybir.AluOpType.min
    nb = rows // P
    nq = ncols // QW

    nc = bacc.Bacc(None, target_bir_lowering=False)

    aug_t_d = nc.dram_tensor("aug_t", [K_AUG, rows], bf16, kind="ExternalInput")
    aug_x_d = nc.dram_tensor("aug_x", [K_AUG, ncols], bf16, kind="ExternalInput")
    dist1_d = nc.dram_tensor(
        "dist1", [P, 2 * nb + 2], f32, kind="ExternalOutput"
    )
    colacc_d = nc.dram_tensor(
        "colacc", [P, (1 + E_RAW) * ncols], bf16, kind="ExternalOutput"
    )

    with TileContext(nc) as tc, ExitStack() as ctx:
        singles = ctx.enter_context(tc.tile_pool(name="singles", bufs=1))
        psum_pool = ctx.enter_context(
            tc.tile_pool(name="psum_pool", bufs=2, space="PSUM")
        )
        dpool = ctx.enter_context(tc.tile_pool(name="dpool", bufs=4))
        spool = ctx.enter_context(tc.tile_pool(name="spool", bufs=2))

        # inputs split into chunked tiles so the first matmuls only wait on
        # the first chunks' DMAs.
        TCH = 8 * P
        aug_t_sb = [
            singles.tile([K_AUG, TCH], bf16, name=f"aug_t_{c}")
            for c in range(rows // TCH)
        ]
        aug_x_sb = [
            singles.tile([K_AUG, QW], bf16, name=f"aug_x_{q}") for q in range(nq)
        ]
        nc.gpsimd.dma_start(out=aug_t_sb[0], in_=aug_t_d[:, :TCH])
        nc.sync.dma_start(out=aug_x_sb[0], in_=aug_x_d[:, :QW])
        for c in range(1, rows // TCH):
            nc.gpsimd.dma_start(
                out=aug_t_sb[c], in_=aug_t_d[:, c * TCH : (c + 1) * TCH]
            )
        for q in range(1, nq):
            nc.sync.dma_start(
                out=aug_x_sb[q], in_=aug_x_d[:, q * QW : (q + 1) * QW]
            )
        colacc = singles.tile([P, ncols], bf16)
        # per-block row-mins, min-folded on host:
        #   col b:    big TS over the ScalarE-drained head of block b
        #   col nb+b: V-drained tail (fused TS over [ncols-VW:] for normal
        #             blocks, the whole q2 for raw blocks)
        #   cols 2nb, 2nb+1: last block's q1/q2 (consumed inline)
        rowmin = singles.tile([P, 2 * nb + 2], f32)

        def emit_ts(b, dst):
            """VectorE row-min over the ScalarE-drained part of a block."""
            hw_ = 3 * QW if b >= nb - E_RAW else ncols - VW
            scr = spool.tile([P, hw_], bf16, tag="scr", name=f"scr_{b}")
            nc.vector.tensor_scalar(
                scr,
                dst[:, :hw_],
                FLT_BIG,
                None,
                op0=MIN,
                op1=MIN,
                accum_out=rowmin[:, b : b + 1],
            )

        def emit_tt(b, dst):
            """VectorE col-min merge of a fully drained block into colacc."""
            if b == 0 or b >= nb - E_RAW:
                return
            nc.vector.tensor_tensor(colacc, colacc, dst, MIN)

        prev = None
        for b in range(nb):
            last = b == nb - 1
            raw = b >= nb - E_RAW
            if b == 0:
                dst = colacc
            else:
                dst = dpool.tile([P, ncols], bf16, tag="dt", name=f"dt_{b}")
            if prev is not None:
                emit_ts(b - 1, prev)
            if last:
                # colacc is final once the last merged block's TT retired
                # (program-ordered after emit_tt(nb-1-E_RAW)); chunked so the
                # serial DMA resource starts moving it as early as possible
                for q in range(nq):
                    nc.gpsimd.dma_start(
                        out=colacc_d[:, q * QW : (q + 1) * QW],
                        in_=colacc[:, q * QW : (q + 1) * QW],
                    )
            for q in range(nq):
                ps = psum_pool.tile([P, QW], f32, tag="ps", name=f"ps_{b}_{q}")
                tch = (b * P) // TCH
                toff = (b * P) % TCH
                for h in range(QW // MM_N):
                    nc.tensor.matmul(
                        ps[:, h * MM_N : (h + 1) * MM_N],
                        lhsT=aug_t_sb[tch][:, toff : toff + P],
                        rhs=aug_x_sb[q][:, h * MM_N : (h + 1) * MM_N],
                        start=True,
                        stop=True,
                    )
                dq = dst[:, q * QW : (q + 1) * QW]
                if raw and q == nq - 1:
                    # raw blocks: VectorE drains all of q3 (fused with its
                    # row-min) — one V op at the tail instead of an ACT
                    # drain followed by a dependent V row-min
                    nc.vector.tensor_scalar(
                        dq,
                        ps,
                        FLT_BIG,
                        None,
                        op0=MIN,
                        op1=MIN,
                        accum_out=rowmin[:, nb + b : nb + b + 1],
                    )
                elif not raw and q == nq - 1:
                    # split drain: ScalarE takes the head, VectorE the tail
                    # (fused with the tail's row-min accumulation)
                    nc.scalar.activation(
                        dq[:, : QW - VW],
                        ps[:, : QW - VW],
                        mybir.ActivationFunctionType.Copy,
                    )
                    nc.vector.tensor_scalar(
                        dq[:, QW - VW :],
                        ps[:, QW - VW :],
                        FLT_BIG,
                        None,
                        op0=MIN,
                        op1=MIN,
                        accum_out=rowmin[:, nb + b : nb + b + 1],
                    )
                else:
                    nc.scalar.activation(
                        dq, ps, mybir.ActivationFunctionType.Copy
                    )
                    if last:
                        # last block's q0-q2 row-mins inline per quarter so
                        # they overlap the remaining drains
                        acc = b if q == 0 else 2 * nb + (q - 1)
                        scr = spool.tile(
                            [P, QW], bf16, tag="scrq", name=f"scrq_{q}"
                        )
                        nc.vector.tensor_scalar(
                            scr,
                            dq,
                            FLT_BIG,
                            None,
                            op0=MIN,
                            op1=MIN,
                            accum_out=rowmin[:, acc : acc + 1],
                        )
                if raw:
                    # col-min contribution merged on host: ship raw quarters
                    roff = (1 + b - (nb - E_RAW)) * ncols
                    dmaq = nc.sync if q % 2 == 0 else nc.gpsimd
                    dmaq.dma_start(
                        out=colacc_d[:, roff + q * QW : roff + (q + 1) * QW],
                        in_=dq,
                    )
            if prev is not None and not last:
                emit_tt(b - 1, prev)
            prev = dst
        nc.sync.dma_start(out=dist1_d[:, :], in_=rowmin)

    return nc


def _get_nc(rows, ncols):
    key = (rows, ncols)
    if key not in _NC_CACHE:
        nc = _build(rows, ncols)
        nc.compile()
        _NC_CACHE[key] = nc
    return _NC_CACHE[key]


def _split_hi_lo(v):
    import ml_dtypes

    hi = v.astype(ml_dtypes.bfloat16)
    lo = (v - hi.astype(np.float32)).astype(ml_dtypes.bfloat16)
    return hi, lo


def _make_aug(t, x):
    """t: [R,3] f32, x: [N,3] f32 -> (aug_t [13,R] bf16, aug_x [13,N] bf16).

    d = sum_k aug_t[k].T * aug_x[k]:
      k0-2 : hi_t  *  hi_w      (w = -2x)
      k3-5 : lo_t  *  hi_w
      k6-8 : hi_t  *  lo_w
      k9   : nth   *  1         (nt = |t|^2 = nth + ntl)
      k10  : ntl   *  1
      k11  : 1     *  nxh       (nx = |x|^2 = nxh + nxl)
      k12  : 1     *  nxl
    """
    import ml_dtypes

    bf = ml_dtypes.bfloat16
    R = t.shape[0]
    N = x.shape[0]
    w = -2.0 * x
    ht, lt = _split_hi_lo(t.T)  # [3, R]
    hw, lw = _split_hi_lo(w.T)  # [3, N]
    nt = (t.astype(np.float64) ** 2).sum(1).astype(np.float32)
    nx = (x.astype(np.float64) ** 2).sum(1).astype(np.float32)
    nth, ntl = _split_hi_lo(nt)
    nxh, nxl = _split_hi_lo(nx)

    aug_t = np.empty((K_AUG, R), bf)
    aug_t[0:3] = ht
    aug_t[3:6] = lt
    aug_t[6:9] = ht
    aug_t[9] = nth
    aug_t[10] = ntl
    aug_t[11] = bf(1.0)
    aug_t[12] = bf(1.0)

    aug_x = np.empty((K_AUG, N), bf)
    aug_x[0:3] = hw
    aug_x[3:6] = hw
    aug_x[6:9] = lw
    aug_x[9] = bf(1.0)
    aug_x[10] = bf(1.0)
    aug_x[11] = nxh
    aug_x[12] = nxl
    return aug_t, aug_x


def _make_in_maps(tp, xh):
    B, N, _ = tp.shape
    half = N // 2
    aug_xs = [_make_aug(tp[b, :1], xh[b])[1] for b in range(B)]
    in_maps = []
    for c in range(N_CORES):
        bidx, h = divmod(c, 2)
        t = tp[bidx, h * half : (h + 1) * half]  # [half, 3]
        aug_t, _ = _make_aug(t, xh[bidx, :1])
        in_maps.append({"aug_t": aug_t, "aug_x": aug_xs[bidx]})
    return in_maps


def _combine(results, B, N):
    d1_sum = 0.0
    d2_sum = 0.0
    for bidx in range(B):
        ccs = []
        for h in range(2):
            r = results[2 * bidx + h]
            d1 = np.asarray(r["dist1"]).astype(np.float64)
            nb = (d1.shape[1] - 2) // 2
            m = np.minimum(d1[:, :nb], d1[:, nb : 2 * nb])
            # the last block's q1/q2 mins live in the extra columns
            m[:, nb - 1] = np.minimum(m[:, nb - 1], d1[:, 2 * nb :].min(1))
            d1_sum += float(m.sum())
            cc = np.asarray(r["colacc"]).astype(np.float32)
            N = cc.shape[1] // (1 + E_RAW)
            # fold raw-exported blocks into the col-min partial
            ccs.append(cc.reshape(cc.shape[0], 1 + E_RAW, N).min(axis=1))
        m = np.minimum(ccs[0], ccs[1]).min(axis=0)
        d2_sum += float(m.astype(np.float64).sum())
    return np.float32(d1_sum / (B * N) + d2_sum / (B * N))


def _run(inputs, trace=False):
    tp = np.ascontiguousarray(np.asarray(inputs["target_pos"], np.float32))
    xh = np.ascontiguousarray(np.asarray(inputs["x_hat"], np.float32))
    B, N, _ = tp.shape
    half = N // 2
    in_maps = _make_in_maps(tp, xh)
    nc = _get_nc(half, N)
    from concourse.bass_utils import run_bass_kernel_spmd

    res = run_bass_kernel_spmd(
        nc, in_maps, list(range(N_CORES)), trace=trace
    )
    loss = _combine(res.results, B, N)
    return loss, res


def kernel(**inputs) -> np.ndarray:
    loss, _ = _run(inputs)
    return loss
